# revision 6
# baseline (speedup 1.0000x reference)
"""Causal multi-head attention (B=128, T=256, C=384, H=6, Dh=64) on 8 TRN2
NeuronCores, data-parallel over batch (16 batches per core, no collectives).

Layout strategy per core:
  - host pre-transposes x to xT [b, C, T] and casts activations/weights to bf16
  - QT/KT computed as [D, T] (Dh on partitions) so scores = QT_h.T @ KT_h needs
    no on-chip transpose of Q/K
  - V computed as [T, D] so AV contraction (over key positions) has keys on
    partitions
  - softmax over the free dim (keys) without max-subtraction (scores are
    O(10) here, exp cannot overflow in fp32); row sums fused into the exp
    activation via accum_out
  - P is transposed on the PE (bf16, 1 cycle/row) for the AV matmul
  - output projection consumes OT [D, T] as the stationary operand directly
"""

import sys

sys.path.insert(0, "/opt/trn_rl_repo")

import numpy as np
import ml_dtypes

import concourse.bass as bass
import concourse.tile as tile
from concourse import mybir
from concourse.bass_utils import run_bass_kernel_spmd
from concourse.masks import make_causal_mask, make_identity

def split_multi_waits(nc):
    """This walrus build accepts at most one sync-wait command per
    instruction; hoist extra waits into standalone InstEventSemaphore
    instructions on the same engine queue (queue waits run in order before
    the original instruction, so semantics are preserved)."""
    ctr = [0]

    def mk(engine, wait):
        ctr[0] += 1
        return mybir.InstEventSemaphore(
            name=f"WSPLIT-{ctr[0]}",
            engine=engine,
            ins=[],
            outs=[],
            sync_info=mybir.SyncInfo(on_wait=[wait], on_update=[]),
        )

    for f in nc.m.functions:
        for blk in f.blocks:
            insts = blk.instructions
            out = []
            for inst in insts:
                si = inst.sync_info
                if si is not None and len(si.on_wait) > 1:
                    waits = list(si.on_wait)
                    for w in waits[:-1]:
                        out.append(mk(inst.engine, w))
                    inst.sync_info = mybir.SyncInfo(
                        on_wait=[waits[-1]], on_update=list(si.on_update)
                    )
                out.append(inst)
            insts[:] = out
    return nc


N_CORES = 8
B, T, C = 128, 256, 384
H, DH = 6, 64
BL = B // N_CORES  # batches per core
BF16 = mybir.dt.bfloat16
FP32 = mybir.dt.float32
AFT = mybir.ActivationFunctionType
SCALE = DH**-0.5  # 0.125
NEG = -1.0e9


def build_kernel() -> bass.Bass:
    nc = bass.Bass()
    xT = nc.dram_tensor("xT", [BL, C, T], BF16, kind="ExternalInput")
    wqt = nc.dram_tensor("wqt", [C, C], BF16, kind="ExternalInput")  # Wq.T [C, D]
    wkt = nc.dram_tensor("wkt", [C, C], BF16, kind="ExternalInput")
    wvt = nc.dram_tensor("wvt", [C, C], BF16, kind="ExternalInput")
    wot = nc.dram_tensor("wot", [C, C], BF16, kind="ExternalInput")  # Wo.T [D, C]
    y = nc.dram_tensor("y", [BL, T, C], FP32, kind="ExternalOutput")

    with tile.TileContext(nc) as tc:
        with (
            tc.tile_pool(name="const", bufs=1) as const,
            tc.tile_pool(name="xp", bufs=3) as xp,
            tc.tile_pool(name="qkv", bufs=2) as qkv,
            tc.tile_pool(name="pp", bufs=3) as pp,
            tc.tile_pool(name="ptp", bufs=3) as ptp,
            tc.tile_pool(name="st", bufs=4) as st,
            tc.tile_pool(name="otp", bufs=2) as otp,
            tc.tile_pool(name="yp", bufs=3) as yp,
            tc.tile_pool(name="psA", bufs=2, space="PSUM") as psA,
            tc.tile_pool(name="psS", bufs=4, space="PSUM") as psS,
            tc.tile_pool(name="psO", bufs=2, space="PSUM") as psO,
        ):
            ident = const.tile([128, 128], BF16)
            make_identity(nc, ident)
            causal = const.tile([128, 128], FP32)
            make_causal_mask(nc, causal, mask_val=NEG)
            # full-row mask for the second query block: [zeros | causal]
            mask1 = const.tile([128, T], FP32)
            nc.vector.memset(mask1[:, 0:128], 0.0)
            nc.vector.tensor_copy(mask1[:, 128:T], causal)

            w_sb = {}
            for name, dram in (("wq", wqt), ("wk", wkt), ("wv", wvt), ("wo", wot)):
                w = const.tile([128, 3, C], BF16, tag=name)
                nc.sync.dma_start(out=w, in_=dram.rearrange("(k p) d -> p k d", p=128))
                w_sb[name] = w

            for b in range(BL):
                # ---- load xT[b] : [C, T] as 3 partition chunks ----
                xt = xp.tile([128, 3, T], BF16)
                nc.sync.dma_start(out=xt, in_=xT[b].rearrange("(k p) t -> p k t", p=128))

                # ---- QT/KT = [D, T] ----
                qt = qkv.tile([128, 3, T], BF16, tag="qt")
                kt = qkv.tile([128, 3, T], BF16, tag="kt")
                for dst, wname in ((qt, "wq"), (kt, "wk")):
                    w = w_sb[wname]
                    for d in range(3):
                        ps = psA.tile([128, C], FP32, tag="big")
                        for k in range(3):
                            nc.tensor.matmul(
                                ps[:, 0:T],
                                lhsT=w[:, k, d * 128 : (d + 1) * 128],
                                rhs=xt[:, k, :],
                                start=(k == 0),
                                stop=(k == 2),
                            )
                        nc.any.tensor_copy(dst[:, d, :], ps[:, 0:T])

                # ---- V = [T, D] ----
                v = qkv.tile([128, 2, C], BF16, tag="v")
                for t2 in range(2):
                    ps = psA.tile([128, C], FP32, tag="big")
                    for k in range(3):
                        nc.tensor.matmul(
                            ps,
                            lhsT=xt[:, k, t2 * 128 : (t2 + 1) * 128],
                            rhs=w_sb["wv"][:, k, :],
                            start=(k == 0),
                            stop=(k == 2),
                        )
                    nc.any.tensor_copy(v[:, t2, :], ps)

                # ---- attention heads ----
                ot = otp.tile([128, 3, T], BF16)  # OT [D, T]
                for pair in range(3):
                    po = psO.tile([128, T], FP32)
                    for sub in range(2):
                        h = 2 * pair + sub
                        doff = sub * 64
                        qh = qt[doff : doff + 64, pair, :]
                        kh = kt[doff : doff + 64, pair, :]

                        # scores (skip fully masked upper-right block)
                        s0 = psS.tile([128, 128], FP32, tag="ps256")
                        s1 = psS.tile([128, T], FP32, tag="ps256")
                        nc.tensor.matmul(
                            s0, lhsT=qh[:, 0:128], rhs=kh[:, 0:128], start=True, stop=True
                        )
                        nc.tensor.matmul(
                            s1, lhsT=qh[:, 128:T], rhs=kh, start=True, stop=True
                        )

                        # mask + exp + row-sum
                        sm0 = st.tile([128, 128], FP32, tag="sm0")
                        sm1 = st.tile([128, T], FP32, tag="sm1")
                        nc.vector.tensor_add(sm0, s0, causal)
                        nc.vector.tensor_add(sm1, s1, mask1)

                        p0 = pp.tile([128, 128], BF16, tag="p0")
                        p1 = pp.tile([128, T], BF16, tag="p1")
                        sums = st.tile([128, 2], FP32, tag="sums")
                        rs = st.tile([128, 2], FP32, tag="rs")
                        nc.scalar.activation(
                            p0, sm0, AFT.Exp, scale=SCALE, accum_out=sums[:, 0:1]
                        )
                        nc.scalar.activation(
                            p1, sm1, AFT.Exp, scale=SCALE, accum_out=sums[:, 1:2]
                        )
                        nc.vector.reciprocal(rs, sums)
                        nc.vector.tensor_scalar_mul(p0, p0, rs[:, 0:1])
                        nc.vector.tensor_scalar_mul(p1, p1, rs[:, 1:2])

                        # transpose P blocks: PT[ts, tq]
                        pt = ptp.tile([128, 2, T], BF16)
                        for ts_, tq_, src in ((0, 0, p0), (0, 1, p1[:, 0:128]), (1, 1, p1[:, 128:T])):
                            tp = psS.tile([128, 128], BF16, tag="ps256")
                            nc.tensor.transpose(tp, src, ident)
                            nc.any.tensor_copy(
                                pt[:, ts_, tq_ * 128 : (tq_ + 1) * 128], tp
                            )

                        # AV: OT_h [64, T] = sum_ts V_h[ts].T @ PT[ts]
                        out_ap = po[doff : doff + 64, :]
                        nc.tensor.matmul(
                            out_ap[:, 0:128],
                            lhsT=v[:, 0, h * 64 : (h + 1) * 64],
                            rhs=pt[:, 0, 0:128],
                            start=True,
                            stop=True,
                            tile_position=(0, doff),
                        )
                        for ts_ in range(2):
                            nc.tensor.matmul(
                                out_ap[:, 128:T],
                                lhsT=v[:, ts_, h * 64 : (h + 1) * 64],
                                rhs=pt[:, ts_, 128:T],
                                start=(ts_ == 0),
                                stop=(ts_ == 1),
                                tile_position=(0, doff),
                            )
                    nc.any.tensor_copy(ot[:, pair, :], po)

                # ---- y = OT.T @ WoT : [T, C] ----
                for t2 in range(2):
                    ps = psA.tile([128, C], FP32, tag="big")
                    for k in range(3):
                        nc.tensor.matmul(
                            ps,
                            lhsT=ot[:, k, t2 * 128 : (t2 + 1) * 128],
                            rhs=w_sb["wo"][:, k, :],
                            start=(k == 0),
                            stop=(k == 2),
                        )
                    ys = yp.tile([128, C], FP32)
                    nc.any.tensor_copy(ys, ps)
                    nc.sync.dma_start(out=y[b, t2 * 128 : (t2 + 1) * 128, :], in_=ys)
    return nc


_NC = None


def _get_nc():
    global _NC
    if _NC is None:
        _NC = split_multi_waits(build_kernel())
    return _NC


def kernel(x, Wq, Wk, Wv, Wo, _trace=False):
    bf16 = ml_dtypes.bfloat16
    wq_t = np.ascontiguousarray(Wq.T).astype(bf16)
    wk_t = np.ascontiguousarray(Wk.T).astype(bf16)
    wv_t = np.ascontiguousarray(Wv.T).astype(bf16)
    wo_t = np.ascontiguousarray(Wo.T).astype(bf16)
    in_maps = []
    for i in range(N_CORES):
        xs = x[i * BL : (i + 1) * BL]  # [BL, T, C]
        xs_t = np.ascontiguousarray(xs.transpose(0, 2, 1)).astype(bf16)
        in_maps.append(
            {"xT": xs_t, "wqt": wq_t, "wkt": wk_t, "wvt": wv_t, "wot": wo_t}
        )
    res = run_bass_kernel_spmd(
        _get_nc(), in_maps, list(range(N_CORES)), trace=_trace
    )
    out = np.concatenate([r["y"] for r in res.results], axis=0)
    if _trace:
        return out.astype(np.float32), res
    return out.astype(np.float32)
